# revision 18
# baseline (speedup 1.0000x reference)
# Trainium2 Bass kernel for nn_MultiCondLayer — Strassen level-1.
#   out = W'@x + b' (W' = sum_k W[k], b' = sum_k b[k]), mask applied on host.
#
# Strassen over 2x2 blocks (A = W' [2x2 of 512], B = x [2x2 of 512c x 2048n]):
#   M1=(A11+A22)(B11+B22) M2=(A21+A22)B11 M3=A11(B12-B22) M4=A22(B21-B11)
#   M5=(A11+A12)B22 M6=(A21-A11)(B11+B12) M7=(A12-A22)(B21+B22)
#   C11=M1+M4-M5+M7  C12=M3+M5  C21=M2+M4  C22=M1-M2+M3+M6
# 448 PE matmuls instead of 512 (-12.5% on the PE-bound critical path).
# A-combos are free on the host; B-combos are wide DVE/GpSimd tensor_tensor
# ops; C-recombination is folded into PSUM evictions via scalar_tensor_tensor
# (bias rides the stt scalar port). Host-simulated rel err 6.95e-3 (gate 2e-2).
#
# Work is split into n-halves h=0,1: block-0 cols h*1024.. and block-1 cols
# 2048+h*1024.. (x superchunks sc_h and sc_{2+h}). Per half: 7 Ms x 8 psum
# banks x 4 c-chunk matmuls. M order M2,M5,M4,M1,M7,M3,M6 so each C block
# completes (and stores) as early as possible.
#
# Engine split: stt evictions on DVE (GpSimd can't read PSUM); plain copy
# evictions on ACT (activation Identity); SBUF-only recombine tensor_tensors
# on GpSimd; B-combos round-robin DVE/GpSimd.

import numpy as np
import ml_dtypes

import concourse.bass as bass
import concourse.mybir as mybir
import concourse.tile as tile
from concourse import bacc
from concourse.bass_utils import run_bass_kernel_spmd

P = 128
B, C, N = 8, 1024, 4096
O = 1024
NT = 512
CO, OO = C // P, O // P
F32 = mybir.dt.float32
BF16 = mybir.dt.bfloat16
ADD = mybir.AluOpType.add
SUB = mybir.AluOpType.subtract
IDENT = mybir.ActivationFunctionType.Identity

N_CORES = 8
H = 512                  # block size (o and c)
NB = 2048                # n block size
NH = 512                 # n quarter width per block
NSUB = NH // NT          # 1


def build_module():
    nc = bacc.Bacc("TRN2", target_bir_lowering=False, debug=False,
                   num_devices=N_CORES)
    x = nc.dram_tensor("x", [C, N], BF16, kind="ExternalInput")
    # 7 pre-combined stationary operands, each [512c, 512o] (A_m.T)
    wts = nc.dram_tensor("wts", [7, H, H], BF16, kind="ExternalInput")
    bv = nc.dram_tensor("bv", [P, OO], F32, kind="ExternalInput")
    out = nc.dram_tensor("out", [O, N], BF16, kind="ExternalOutput")

    x_r = x.ap().rearrange("(c p) n -> p c n", p=P)        # [128, 8, N]
    w_r = wts.ap().rearrange("m (c p) o -> p m c o", p=P)  # [128, 7, 4, 512]
    out_r = out.ap().rearrange("(oo p) n -> p oo n", p=P)  # [128, 8, N]

    with tile.TileContext(nc) as tc:
        with (
            tc.tile_pool(name="consts", bufs=1) as consts,
            tc.tile_pool(name="xs", bufs=4) as xs,
            tc.tile_pool(name="cmbs", bufs=2) as cmbs,
            tc.tile_pool(name="ms", bufs=7) as ms,
            tc.tile_pool(name="tmps", bufs=8) as tmps,
            tc.tile_pool(name="outs", bufs=10) as outs,
            tc.tile_pool(name="ps", bufs=8, space="PSUM") as psp,
        ):
            wones = consts.tile([P, P], BF16)
            nc.vector.memset(wones[:], 0.125)
            xones = consts.tile([P, NT], BF16)
            nc.vector.memset(xones[:], 0.125)
            for i in range(4):
                wps = psp.tile([P, NT], F32, name=f"warm_{i}", tag="ps")
                nc.tensor.matmul(wps[:], wones[:], xones[:],
                                 start=True, stop=True)

            w_sb = consts.tile([P, 7, 4, H], BF16)
            bias_sb = consts.tile([P, OO], F32)
            # first matmul gates on M2's (m=1) c0 / o0:128 quarter
            nc.scalar.dma_start(w_sb[:, 1, 0, 0:P], w_r[:, 1, 0, 0:P])
            nc.scalar.dma_start(w_sb[:, 1, 0, P:H], w_r[:, 1, 0, P:H])
            nc.scalar.dma_start(w_sb[:, 1, 1:4, :], w_r[:, 1, 1:4, :])
            nc.scalar.dma_start(bias_sb[:], bv.ap())
            for m in (4, 3, 0, 6, 2, 5):   # M5, M4, M1, M7, M3, M6 order
                nc.scalar.dma_start(w_sb[:, m, :, :], w_r[:, m, :, :])

            # x superchunk tiles [128, 8c, 1024]; sc0/sc2 feed half 0,
            # sc1/sc3 feed half 1.
            x_sc = {}

            def load_x(h):
                # emitted lazily (inside quarter h-2) so pool-slot waits do
                # not head-of-line-block the stores on the Sync queue
                sa = h * NH       # block-0 col offset
                sb = NB + h * NH  # block-1 col offset
                xa = xs.tile([P, CO, NH], BF16, name=f"xa_{h}", tag="xs")
                xb = xs.tile([P, CO, NH], BF16, name=f"xb_{h}", tag="xs")
                if h == 0:
                    # fine-grained first quarter (M2 consumes c0..c3 in order)
                    for c in range(CO):
                        nc.sync.dma_start(xa[:, c, :], x_r[:, c, 0:NH])
                    # M5 needs xb c4..7 first
                    nc.sync.dma_start(xb[:, 4:8, :], x_r[:, 4:8, sb:sb + NH])
                    nc.sync.dma_start(xb[:, 0:4, :], x_r[:, 0:4, sb:sb + NH])
                else:
                    nc.sync.dma_start(xa[:], x_r[:, :, sa:sa + NH])
                    nc.sync.dma_start(xb[:], x_r[:, :, sb:sb + NH])
                x_sc[h] = (xa, xb)

            load_x(0)
            load_x(1)

            for h in range(4):
                xa, xb = x_sc[h]
                n0a = h * NH          # block-0 col offset in out
                n0b = NB + h * NH     # block-1 col offset in out


                # SBUF M tiles we must retain (M6/M7 are consumed at evict)
                mt = {i: ms.tile([P, 4, NH], BF16, name=f"m{i}_{h}", tag="m")
                      for i in (0, 1, 2, 3, 4, 5)}  # M1..M5 + M6

                def combo(idx, s0, sl0, s1, sl1, op, eng):
                    cm = cmbs.tile([P, 4, NH], BF16,
                                   name=f"cmb{idx}_{h}", tag="cmb")
                    eng.tensor_tensor(cm[:], s0[:, sl0, :], s1[:, sl1, :],
                                      op=op)
                    return cm

                def mm(m, rhs_tile, rhs_base, pst):
                    # one M: 8 banks, cc-inner; returns dict of psum tiles
                    pss = {}
                    for oc in range(4):
                        for nsub in range(NSUB):
                            ps = psp.tile([P, NT], F32,
                                          name=f"ps_{h}_{pst}_{oc}_{nsub}",
                                          tag="ps")
                            pss[oc, nsub] = ps
                            for cc in range(4):
                                nc.tensor.matmul(
                                    ps[:],
                                    w_sb[:, m, cc, oc * P:(oc + 1) * P],
                                    rhs_tile[:, rhs_base + cc,
                                             nsub * NT:(nsub + 1) * NT],
                                    start=(cc == 0), stop=(cc == 3),
                                )
                    return pss

                def evict_copy(pss, dst):
                    # plain copy eviction on ACT (psum -> sbuf bf16)
                    for oc in range(4):
                        for nsub in range(NSUB):
                            nc.scalar.activation(
                                dst[:, oc, nsub * NT:(nsub + 1) * NT],
                                pss[oc, nsub][:], IDENT)

                # ---- M2 = A'(m=1) @ B11 ----
                ps2 = mm(1, xa, 0, "m2")
                evict_copy(ps2, mt[1])
                if h + 2 < 4:
                    load_x(h + 2)
                # ---- M5 = A'(m=4) @ B22 ----
                ps5 = mm(4, xb, 4, "m5")
                evict_copy(ps5, mt[4])
                # ---- M4 = A'(m=3) @ (B21-B11) ----
                cm4 = combo(4, xa, slice(4, 8), xa, slice(0, 4), SUB,
                            nc.vector)
                ps4 = mm(3, cm4, 0, "m4")
                # C21 = (M4 + bias1) + M2, done at M4's eviction (DVE stt);
                # M4 also copied for C11 (ACT).
                for oc in range(4):
                    otc21 = outs.tile([P, NSUB, NT], BF16,
                                      name=f"c21_{h}_{oc}", tag="ot")
                    for nsub in range(NSUB):
                        nc.vector.scalar_tensor_tensor(
                            otc21[:, nsub, :], ps4[oc, nsub][:],
                            bias_sb[:, 4 + oc:5 + oc],
                            mt[1][:, oc, nsub * NT:(nsub + 1) * NT],
                            op0=ADD, op1=ADD)
                        nc.scalar.activation(
                            mt[3][:, oc, nsub * NT:(nsub + 1) * NT],
                            ps4[oc, nsub][:], IDENT)
                    nc.sync.dma_start(
                        out_r[:, 4 + oc, n0a:n0a + NH], otc21[:])
                # ---- M1 = A'(m=0) @ (B11+B22) ----
                cm1 = combo(1, xa, slice(0, 4), xb, slice(4, 8), ADD,
                            nc.vector)
                ps1 = mm(0, cm1, 0, "m1")
                evict_copy(ps1, mt[0])
                # pre-combined partials, off the PE critical path:
                #   t1 = M1+M4 (GpSimd, slack until M7), t1b = t1-M5 (DVE)
                #   t3 = M1-M2 (GpSimd, slack until M6)
                t1b = {}
                t3 = {}
                for oc in range(4):
                    for nsub in range(NSUB):
                        sl = slice(nsub * NT, (nsub + 1) * NT)
                        t1 = tmps.tile([P, NT], BF16,
                                       name=f"t1_{h}_{oc}_{nsub}", tag="tmp")
                        nc.gpsimd.tensor_tensor(
                            t1[:], mt[0][:, oc, sl], mt[3][:, oc, sl], op=ADD)
                        tb = tmps.tile([P, NT], BF16,
                                       name=f"t1b_{h}_{oc}_{nsub}", tag="tmp")
                        nc.gpsimd.tensor_tensor(
                            tb[:], t1[:], mt[4][:, oc, sl], op=SUB)
                        t1b[oc, nsub] = tb
                        tt3 = tmps.tile([P, NT], BF16,
                                        name=f"t3_{h}_{oc}_{nsub}", tag="tmp")
                        nc.gpsimd.tensor_tensor(
                            tt3[:], mt[0][:, oc, sl], mt[1][:, oc, sl],
                            op=SUB)
                        t3[oc, nsub] = tt3
                # ---- M7 = A'(m=6) @ (B21+B22) ----
                cm7 = combo(7, xa, slice(4, 8), xb, slice(4, 8), ADD,
                            nc.vector)
                ps7 = mm(6, cm7, 0, "m7")
                # C11 = (M7 + bias0) + (M1+M4-M5): completes at eviction
                for oc in range(4):
                    otc11 = outs.tile([P, NSUB, NT], BF16,
                                      name=f"c11_{h}_{oc}", tag="ot")
                    for nsub in range(NSUB):
                        nc.vector.scalar_tensor_tensor(
                            otc11[:, nsub, :], ps7[oc, nsub][:],
                            bias_sb[:, oc:oc + 1], t1b[oc, nsub][:],
                            op0=ADD, op1=ADD)
                    nc.sync.dma_start(
                        out_r[:, oc, n0a:n0a + NH], otc11[:])
                # ---- M3 = A'(m=2) @ (B12-B22) ----
                cm3 = combo(3, xb, slice(0, 4), xb, slice(4, 8), SUB,
                            nc.vector)
                ps3 = mm(2, cm3, 0, "m3")
                # C12 = (M3 + bias0) + M5 at eviction; M3 copied for C22;
                # t5 = (M1-M2) + M3 (DVE, slack until M6)
                t5 = {}
                for oc in range(4):
                    otc12 = outs.tile([P, NSUB, NT], BF16,
                                      name=f"c12_{h}_{oc}", tag="ot")
                    for nsub in range(NSUB):
                        sl = slice(nsub * NT, (nsub + 1) * NT)
                        nc.vector.scalar_tensor_tensor(
                            otc12[:, nsub, :], ps3[oc, nsub][:],
                            bias_sb[:, oc:oc + 1], mt[4][:, oc, sl],
                            op0=ADD, op1=ADD)
                        nc.scalar.activation(
                            mt[2][:, oc, sl], ps3[oc, nsub][:], IDENT,
                            bias=bias_sb[:, 4 + oc:5 + oc])
                        tt5 = tmps.tile([P, NT], BF16,
                                        name=f"t5_{h}_{oc}_{nsub}", tag="tmp")
                        nc.gpsimd.tensor_tensor(
                            tt5[:], t3[oc, nsub][:], mt[2][:, oc, sl],
                            op=ADD)
                        t5[oc, nsub] = tt5
                    nc.sync.dma_start(
                        out_r[:, oc, n0b:n0b + NH], otc12[:])
                # ---- M6 = A'(m=5) @ (B11+B12) ----
                cm6 = combo(6, xa, slice(0, 4), xb, slice(0, 4), ADD,
                            nc.vector)
                ps6 = mm(5, cm6, 0, "m6")
                # C22 = M6 + (M1-M2+M3+bias1). M6 psum is freed by a
                # fast ACT copy; GpSimd stt assembles from SBUF — keeps the
                # end-of-quarter DVE queue short so next-quarter PSUM frees
                # promptly (4.9us/quarter PE stall otherwise, measured).
                evict_copy(ps6, mt[5])
                for oc in range(4):
                    otc22 = outs.tile([P, NSUB, NT], BF16,
                                      name=f"c22_{h}_{oc}", tag="ot")
                    for nsub in range(NSUB):
                        sl = slice(nsub * NT, (nsub + 1) * NT)
                        nc.vector.tensor_tensor(
                            otc22[:, nsub, :], mt[5][:, oc, sl],
                            t5[oc, nsub][:], op=ADD)
                    nc.sync.dma_start(
                        out_r[:, 4 + oc, n0b:n0b + NH], otc22[:])
    nc.compile()
    return nc


_NC_CACHE = None


def _get_module():
    global _NC_CACHE
    if _NC_CACHE is None:
        _NC_CACHE = build_module()
    return _NC_CACHE


def _make_in_maps(cond, x_mask, W, b):
    Wp = np.asarray(W, dtype=np.float32).sum(axis=0)      # [O, C]
    A11, A12 = Wp[:H, :H], Wp[:H, H:]
    A21, A22 = Wp[H:, :H], Wp[H:, H:]
    combos = [A11 + A22, A21 + A22, A11, A22, A11 + A12,
              A21 - A11, A12 - A22]
    wts = np.ascontiguousarray(
        np.stack([c.T for c in combos], axis=0).astype(ml_dtypes.bfloat16))
    bv = np.ascontiguousarray(
        np.asarray(b, dtype=np.float32).sum(axis=0).reshape(OO, P).T,
        dtype=np.float32)
    in_maps = []
    for core in range(N_CORES):
        in_maps.append({
            "x": np.ascontiguousarray(
                np.asarray(cond[core]).astype(ml_dtypes.bfloat16)),
            "wts": wts,
            "bv": bv,
        })
    return in_maps


def run(cond, x_mask, W, b, trace=False, trace_cores=None):
    nc = _get_module()
    in_maps = _make_in_maps(cond, x_mask, W, b)
    res = run_bass_kernel_spmd(
        nc, in_maps, core_ids=list(range(N_CORES)),
        trace=trace, trace_cores=trace_cores,
    )
    mask = np.asarray(x_mask, dtype=np.float32)
    out = np.stack(
        [np.asarray(res.results[i]["out"]).astype(np.float32)
         for i in range(N_CORES)], axis=0)
    out *= mask
    return out, res


def kernel(cond, x_mask, W, b):
    out, _ = run(cond, x_mask, W, b)
    return out


# revision 19
# speedup vs baseline: 1.3817x; 1.3817x over previous
# Trainium2 Bass kernel for nn_MultiCondLayer:
#   out[b,o,n] = (sum_k (cond[b] @ W[k].T)[o,n] + sum_k b[k,o]) * x_mask[b,0,n]
# Key algebraic reductions:
#  - sum_k Linear_k(x) == Linear(x) with W' = sum_k W[k], b' = sum_k b[k]
#    (4x FLOP reduction vs. the naive einsum over k); W' is summed on host.
#  - The x_mask multiply is a diagonal scale over n, so it commutes with the
#    c-contraction; it is applied EXACTLY on the host to the chip output:
#    (x@W' + b')*mask. The device never touches the mask, which removes the
#    mask DMA + 8 PE broadcast matmuls and makes the PSUM eviction a cheaper
#    2-operand DVE op (measured 658 ns -> ~366 ns per [128,512] tile).
#
# Sharding: data-parallel over batch B=8 across the 8 NeuronCores (one batch
# element per core); the reduced [1024,1024] weight is replicated.
#
# Numerics: x and W' are cast to bf16 on the host; outputs store bf16 and are
# upcast on the host (measured end-to-end rel error ~2.9e-3 vs the 2e-2 gate).
# bf16 matmuls sustain 216 ns/512-free on HW vs fp32r's 233 ns (measured), so
# this is both a PE-rate win (~8%) and an HBM win (36 MB -> 18 MB per core).
#
# Per-core compute: [1024c,4096n] x [1024c,1024o] as 512 PE matmuls
# (128x128 bf16 lhsT, 128x512 bf16 rhs) accumulating in fp32 PSUM, evicted
# by DVE tensor_scalar_add (psum + bias[o]) -> bf16.
#
# Schedule notes (from perfetto traces of prior revs):
#  - ~5 us of fixed engine/queue preamble precedes any DMA; first PE issue
#    is gated on w[og0,c0]+x[c0] only. Warmup matmuls on memset data were
#    tried and REGRESSED (a 1-partition matmul still streams its full free
#    dim at ~440 ns and does not accelerate the DVFS ramp).
#  - x streams alone on the Sync HWDGE queue (superchunk 0 per-c so compute
#    starts early; later superchunks one DMA instruction each); weights,
#    bias and out-stores ride the Activation HWDGE queue.
#  - Matmuls run c-outer/o4/nsub-inner; 8 PSUM banks in flight; evictions
#    chase each group; stores are per-o4 [128,2,512] bf16 (2 KB
#    descriptors).
#  - GpSimd cannot read PSUM (BIR verifier) — all evictions on Vector.

import numpy as np
import ml_dtypes

import concourse.bass as bass
import concourse.mybir as mybir
import concourse.tile as tile
from concourse import bacc
from concourse.bass_utils import run_bass_kernel_spmd

P = 128
B, C, N = 8, 1024, 4096
O = 1024
NT = 512                 # matmul free dim = one fp32 PSUM bank
CO, OO, NN = C // P, O // P, N // NT
F32 = mybir.dt.float32
BF16 = mybir.dt.bfloat16

N_CORES = 8

NSUP = 1024              # n superchunk width (2 KB bf16 DMA descriptors)
NSUPS = N // NSUP        # 4
NSUB = NSUP // NT        # 2 psum-width subchunks per superchunk


def build_module():
    nc = bacc.Bacc("TRN2", target_bir_lowering=False, debug=False,
                   num_devices=N_CORES)
    x = nc.dram_tensor("x", [C, N], BF16, kind="ExternalInput")    # cond[b]
    wt = nc.dram_tensor("wt", [C, O], BF16, kind="ExternalInput")  # (sum_k W[k]).T
    # bias pre-transposed on host to [128, OO] so the DMA is 128 contiguous
    # 32B rows instead of 1024 4-byte gather descriptors.
    bv = nc.dram_tensor("bv", [P, OO], F32, kind="ExternalInput")
    out = nc.dram_tensor("out", [O, N], BF16, kind="ExternalOutput")

    x_r = x.ap().rearrange("(c p) n -> p c n", p=P)      # [128, CO, N]
    wt_r = wt.ap().rearrange("(c p) o -> p c o", p=P)    # [128, CO, O]
    out_r = out.ap().rearrange("(oo p) n -> p oo n", p=P)  # [128, OO, N]

    with tile.TileContext(nc) as tc:
        with (
            tc.tile_pool(name="consts", bufs=1) as consts,
            tc.tile_pool(name="xs", bufs=2) as xs,
            tc.tile_pool(name="outs", bufs=12) as outs,
            tc.tile_pool(name="ps", bufs=8, space="PSUM") as psp,
        ):
            # Full-array warmup: a few matmuls on memset data, gated only on
            # Vector memsets, so the PE's DVFS ramp starts ~5.5 us — while
            # real data DMAs are still gated by the HWDGE queue-init wave
            # (~9 us). Unlike 1-partition warmups (which regressed: no array
            # load, pure occupancy), these light up all 128x128 PEs; worst
            # case they fill PE time that was idle anyway.
            wones = consts.tile([P, P], BF16)
            nc.vector.memset(wones[:], 0.125)
            xones = consts.tile([P, NT], BF16)
            nc.vector.memset(xones[:], 0.125)
            # 4 warmups ≈ what fits between Tensor-seq readiness (~9 us) and
            # first-data-ready (~10.3 us): measured, 8 warmups fully warmed
            # the clock (real matmuls at 216 ns immediately, vs ~310 ns for
            # the first ~10 us without) but the extra 4 delayed real work.
            for i in range(4):
                wps = psp.tile([P, NT], F32, name=f"warm_{i}", tag="ps")
                nc.tensor.matmul(wps[:], wones[:], xones[:],
                                 start=True, stop=True)
            # Weights in per-(o-half, c) chunks: the first matmul is gated by
            # just w[og0,c0]+x[c0]. og0 weights are interleaved with the first
            # superchunk's x chunks; og1 weights follow. (Routing the first
            # chunks via GpSimd SWDGE was tried and REGRESSED ~3 us — SWDGE
            # descriptor building is slower than the HWDGE init wave.)
            OH = O // 2
            w_sb = consts.tile([P, CO, O], BF16)
            bias_sb = consts.tile([P, OO], F32)
            # c0's og0-half split so the very first matmul gates on 32 KB
            nc.scalar.dma_start(w_sb[:, 0, 0:P], wt_r[:, 0, 0:P])
            nc.scalar.dma_start(w_sb[:, 0, P:OH], wt_r[:, 0, P:OH])
            for c in range(1, CO):
                nc.scalar.dma_start(w_sb[:, c, 0:OH], wt_r[:, c, 0:OH])
            nc.scalar.dma_start(bias_sb[:], bv.ap())
            for c in range(CO):
                nc.scalar.dma_start(w_sb[:, c, OH:O], wt_r[:, c, OH:O])

            for ns in range(NSUPS):
                x_sb = xs.tile([P, CO, NSUP], BF16, name=f"x_sb_{ns}",
                               tag="x_sb")
                if ns == 0:
                    # fine-grained so the first matmul starts early (c0 in
                    # halves: the first matmul needs only nsub=0's 512 cols)
                    nc.sync.dma_start(x_sb[:, 0, 0:NT], x_r[:, 0, 0:NT])
                    nc.sync.dma_start(x_sb[:, 0, NT:NSUP], x_r[:, 0, NT:NSUP])
                    for c in range(1, CO):
                        nc.sync.dma_start(
                            x_sb[:, c, :], x_r[:, c, 0:NSUP])
                else:
                    nc.sync.dma_start(
                        x_sb[:], x_r[:, :, ns * NSUP:(ns + 1) * NSUP])
                for og in range(2):
                    # 8 psum groups = 4 o-chunks x 2 n-subchunks; each weight
                    # tile feeds 2 back-to-back matmuls (nsub pair). The very
                    # last group is split into two 4-bank halves so the
                    # end-of-kernel eviction chain is 2 ops per engine
                    # instead of 4 (shorter drain after the final matmul).
                    last = (ns == NSUPS - 1 and og == 1)
                    o4_phases = [(0, 1), (2,), (3,)] if last else [(0, 1, 2, 3)]
                    for phase in o4_phases:
                        pss = {(o4, nsub): psp.tile(
                                   [P, NT], F32,
                                   name=f"ps_{ns}_{og}_{o4}_{nsub}", tag="ps")
                               for o4 in phase for nsub in range(NSUB)}
                        for c in range(CO):
                            for o4 in phase:
                                o = og * 4 + o4
                                for nsub in range(NSUB):
                                    nc.tensor.matmul(
                                        pss[o4, nsub][:],
                                        w_sb[:, c, o * P:(o + 1) * P],
                                        x_sb[:, c, nsub * NT:(nsub + 1) * NT],
                                        start=(c == 0),
                                        stop=(c == CO - 1),
                                    )
                        # Evictions: psum + bias[o] -> bf16, split across
                        # Vector (tensor_scalar_add) and the Activation
                        # engine (Identity with bias AP) — both can read
                        # PSUM; each op is a measured ~660-690 ns, so two
                        # parallel chains halve the end-of-group latency.
                        # Stores: one [128,2,512] bf16 DMA per o4 (2 KB
                        # descriptors) on the Sync queue: the Scalar
                        # sequencer must stay free for ACT evictions (a
                        # DIRECT2D store issue costs 592 ns and was
                        # stretching the tail).
                        half = len(phase) // 2
                        for o4 in phase:
                            o = og * 4 + o4
                            ot = outs.tile([P, NSUB, NT], BF16,
                                           name=f"ot_{ns}_{og}_{o4}", tag="ot")
                            for nsub in range(NSUB):
                                if len(phase) > 1:
                                    on_vector = o4 - phase[0] < half
                                else:
                                    # single-o4 tail phase: split by nsub so
                                    # both engines evict one bank each
                                    on_vector = nsub == 0
                                if on_vector:
                                    nc.vector.tensor_scalar_add(
                                        ot[:, nsub, :], pss[o4, nsub][:],
                                        bias_sb[:, o:o + 1],
                                    )
                                else:
                                    nc.scalar.activation(
                                        ot[:, nsub, :], pss[o4, nsub][:],
                                        mybir.ActivationFunctionType.Identity,
                                        bias=bias_sb[:, o:o + 1],
                                    )
                            # One store per o4 everywhere: per-nsub splitting
                            # in the tail was tried and REGRESSED — the Sync
                            # sequencer serializes DIRECT2D store-issues at
                            # ~600 ns each, costing more than the finer
                            # eviction chasing saved.
                            nc.sync.dma_start(
                                out_r[:, o, ns * NSUP:(ns + 1) * NSUP],
                                ot[:])
    nc.compile()
    return nc


_NC_CACHE = None


def _get_module():
    global _NC_CACHE
    if _NC_CACHE is None:
        _NC_CACHE = build_module()
    return _NC_CACHE


def _make_in_maps(cond, x_mask, W, b):
    wt = np.ascontiguousarray(
        W.sum(axis=0).T.astype(ml_dtypes.bfloat16))                # [C, O] bf16
    bv = np.ascontiguousarray(
        b.sum(axis=0).reshape(OO, P).T, dtype=np.float32)          # [128, OO]
    in_maps = []
    for core in range(N_CORES):
        in_maps.append({
            "x": np.ascontiguousarray(
                np.asarray(cond[core]).astype(ml_dtypes.bfloat16)),
            "wt": wt,
            "bv": bv,
        })
    return in_maps


def run(cond, x_mask, W, b, trace=False, trace_cores=None):
    """Run on hardware; returns (out [B,O,N] fp32, BassKernelResults)."""
    nc = _get_module()
    in_maps = _make_in_maps(cond, x_mask, W, b)
    res = run_bass_kernel_spmd(
        nc, in_maps, core_ids=list(range(N_CORES)),
        trace=trace, trace_cores=trace_cores,
    )
    # The mask multiply commutes with nothing it needs to: it is an exact
    # per-(b,n) diagonal scale applied to the finished (x@W' + b') output.
    mask = np.asarray(x_mask, dtype=np.float32)          # [B, 1, N]
    out = np.stack(
        [np.asarray(res.results[i]["out"]).astype(np.float32)
         for i in range(N_CORES)], axis=0)
    out *= mask
    return out, res


def kernel(cond, x_mask, W, b):
    out, _ = run(cond, x_mask, W, b)
    return out

